# revision 9
# baseline (speedup 1.0000x reference)
"""Trainium2 Bass kernel for nn_AttentionBlock (B=32, C=256, H=W=32).

Data-parallel over batch across 8 NeuronCores (4 batch elements per core);
all parameters replicated.

Algorithm per batch element (x: [C=256, N=1024]):
  h  = GroupNorm(x; 8 groups) * gn_w + gn_b
  q  = (wq/sqrt(C)) @ h + bq/sqrt(C)          [C, N]   (scale folded into wq)
  k  = wk @ h + bk                            [C, N]
  vT = hT @ wvT + 1 x bv                      [N, C]   (produced transposed!)
  ST[j,i] = sum_c k[c,j] q[c,i]               [N, N]   (scores, transposed)
  E  = exp(ST)            (scores are in [-9, 9] for this model; no max-sub)
  rowsum[i] = sum_j E[j,i]                    (ones-vector matmul, PSUM accum)
  outU[c,i] = sum_j vT[j,c] E[j,i]            (PSUM accum over j-tiles)
  y  = x + wp @ (outU * (1/rowsum)) + bp

The transposed-score formulation means no [N,N] transposes are needed:
softmax reductions over j happen on the TensorEngine partition axis via
ones/indicator matmuls, and every big matmul streams N>=256 columns in
fp32r (1 cycle/row).
"""

import numpy as np

import concourse.bacc as bacc
import concourse.bass as bass
import concourse.mybir as mybir
import concourse.tile as tile
from concourse.bass_utils import run_bass_kernel_spmd

B, C, HH, WW = 32, 256, 32, 32
N = HH * WW                 # 1024 spatial positions
NCORES = 8
BPC = B // NCORES           # batch elements per core
G = 8                       # groupnorm groups
GS = C // G                 # channels per group
P = 128                     # SBUF partitions
NCH = C // P                # channel chunks (2)
IH = 512                    # i-half width (fp32 moving-operand max)
NIH = N // IH               # 2
NJ = N // P                 # 8 j-tiles
EPS = 1e-5

F32 = mybir.dt.float32
F32R = mybir.dt.float32r
AF = mybir.ActivationFunctionType
OP = mybir.AluOpType


def r(ap):
    """fp32r APs pass straight through to the TensorEngine."""
    return ap


def build_kernel_body(nc, tc, x_d, y_d, wd, bd, bvr_d, gnw_d, gnb_d, ind_d, indT_d, ones_d):
    ctxpools = dict(
        const=tc.tile_pool(name="const", bufs=1),
        xp=tc.tile_pool(name="xp", bufs=2),
        hp=tc.tile_pool(name="hp", bufs=2),
        qk=tc.tile_pool(name="qk", bufs=2),
        vtp=tc.tile_pool(name="vtp", bufs=2),
        etp=tc.tile_pool(name="etp", bufs=2),
        sm=tc.tile_pool(name="sm", bufs=4),
        outp=tc.tile_pool(name="outp", bufs=2),
        pp=tc.tile_pool(name="pp", bufs=8, space=bass.MemorySpace.PSUM),
    )
    pools = {k: v.__enter__() for k, v in ctxpools.items()}
    const = pools["const"]
    pp = pools["pp"]
    sm = pools["sm"]

    # ---- replicated constants into SBUF ----
    wt = {}   # weights, transposed: [c_chunk][128, 256]
    bt = {}   # per-partition biases: [o_chunk][128, 1]
    for name in ("q", "k", "v", "p"):
        wt[name] = []
        for ch in range(NCH):
            w_tile = const.tile([P, C], F32R, tag=f"w{name}{ch}")
            nc.sync.dma_start(out=w_tile, in_=wd[name][ch * P:(ch + 1) * P, :])
            wt[name].append(w_tile)
    for name in ("q", "k", "p"):
        bt[name] = []
        for ch in range(NCH):
            b_tile = const.tile([P, 1], F32, tag=f"b{name}{ch}")
            nc.sync.dma_start(out=b_tile, in_=bd[name][ch * P:(ch + 1) * P][:, None])
            bt[name].append(b_tile)
    bv_row = const.tile([1, C], F32R, tag="bv_row")
    nc.sync.dma_start(out=bv_row, in_=bvr_d[None, :])

    gnw_t, gnb_t, ind_t, indT_t = [], [], [], []
    for ch in range(NCH):
        gw = const.tile([P, 1], F32, tag=f"gnw{ch}")
        nc.sync.dma_start(out=gw, in_=gnw_d[ch * P:(ch + 1) * P][:, None])
        gnw_t.append(gw)
        gb = const.tile([P, 1], F32, tag=f"gnb{ch}")
        nc.sync.dma_start(out=gb, in_=gnb_d[ch * P:(ch + 1) * P][:, None])
        gnb_t.append(gb)
        it_ = const.tile([P, G], F32, tag=f"ind{ch}")
        nc.sync.dma_start(out=it_, in_=ind_d[ch * P:(ch + 1) * P, :])
        ind_t.append(it_)
        itT = const.tile([G, P], F32, tag=f"indT{ch}")
        nc.sync.dma_start(out=itT, in_=indT_d[:, ch * P:(ch + 1) * P])
        indT_t.append(itT)

    ones_col = const.tile([P, 1], F32R, tag="ones_col")
    nc.sync.dma_start(out=ones_col, in_=ones_d[:, 0:1])
    ones_row = const.tile([1, P], F32R, tag="ones_row")
    nc.sync.dma_start(out=ones_row, in_=ones_d[0:1, :])
    eps8 = const.tile([G, 1], F32, tag="eps8")
    nc.vector.memset(eps8, EPS)

    # ---- per-batch pipeline ----
    for b in range(BPC):
        # load x[b] as two channel-chunk tiles [128, 1024]
        xt = []
        for ch in range(NCH):
            t = pools["xp"].tile([P, N], F32, tag=f"xt{ch}")
            nc.sync.dma_start(out=t, in_=x_d[b, ch * P:(ch + 1) * P, :])
            xt.append(t)

        # -- GroupNorm statistics --
        # per-channel mean / E[x^2] over the 1024 free elements
        pcs = []
        for ch in range(NCH):
            stats = sm.tile([P, 2, 6], F32, tag="bnstats")
            for sg in range(2):
                nc.vector.bn_stats(out=stats[:, sg, :], in_=xt[ch][:, sg * 512:(sg + 1) * 512])
            mv = sm.tile([P, 2], F32, tag="mv")
            nc.vector.bn_aggr(out=mv, in_=stats)
            pc = sm.tile([P, 2], F32, tag=f"pc{ch}")
            nc.vector.tensor_copy(out=pc[:, 0:1], in_=mv[:, 0:1])
            nc.vector.scalar_tensor_tensor(out=pc[:, 1:2], in0=mv[:, 0:1],
                                           scalar=mv[:, 0:1], in1=mv[:, 1:2],
                                           op0=OP.mult, op1=OP.add)  # mean^2 + var
            pcs.append(pc)
        # group-reduce across the 32 channels of each group (partition axis)
        pg = pp.tile([G, 2], F32, tag="ps")
        for ch in range(NCH):
            nc.tensor.matmul(pg, ind_t[ch], pcs[ch], start=(ch == 0), stop=(ch == NCH - 1))
        br8 = sm.tile([G, 2], F32, tag="br8")   # [:,0]=mean_g  [:,1]=rstd_g
        nc.scalar.mul(out=br8, in_=pg, mul=1.0 / 32.0)
        m2g = sm.tile([G, 1], F32, tag="m2g")
        nc.vector.tensor_mul(m2g, br8[:, 0:1], br8[:, 0:1])
        nc.vector.tensor_sub(br8[:, 1:2], br8[:, 1:2], m2g)    # var_g
        nc.scalar.activation(out=br8[:, 1:2], in_=br8[:, 1:2], func=AF.Sqrt, bias=eps8, scale=1.0)
        nc.vector.reciprocal(out=br8[:, 1:2], in_=br8[:, 1:2])

        # broadcast group stats back to channels, fold gn affine, normalize
        ht = []
        for ch in range(NCH):
            pbc = pp.tile([P, 2], F32, tag="ps")
            nc.tensor.matmul(pbc, indT_t[ch], br8)
            s_ = sm.tile([P, 1], F32, tag=f"s{ch}")
            t_ = sm.tile([P, 1], F32, tag=f"t{ch}")
            nc.vector.tensor_mul(s_, pbc[:, 1:2], gnw_t[ch])   # s = rstd * w
            nc.vector.scalar_tensor_tensor(out=t_, in0=pbc[:, 0:1], scalar=s_,
                                           in1=gnb_t[ch], op0=OP.mult,
                                           op1=OP.subtract)    # t = mean*s - b
            h_ = pools["hp"].tile([P, N], F32R, tag=f"ht{ch}")
            nc.vector.tensor_scalar(out=h_, in0=xt[ch], scalar1=s_, scalar2=t_,
                                    op0=OP.mult, op1=OP.subtract)  # x*s - t
            ht.append(h_)

        # -- q, k projections: [C, N] = W^T.T @ h  (+ bias during PSUM move) --
        qt, kt = [], []
        for wname, dst in (("q", qt), ("k", kt)):
            for och in range(NCH):
                d = pools["qk"].tile([P, N], F32R, tag=f"{wname}t{och}")
                dst.append(d)
                for ih in range(NIH):
                    pq = pp.tile([P, IH], F32, tag="ps")
                    for cch in range(NCH):
                        nc.tensor.matmul(
                            pq,
                            r(wt[wname][cch][:, och * P:(och + 1) * P]),
                            r(ht[cch][:, ih * IH:(ih + 1) * IH]),
                            start=(cch == 0), stop=(cch == NCH - 1))
                    nc.vector.tensor_scalar_add(
                        out=d[:, ih * IH:(ih + 1) * IH], in0=pq, scalar1=bt[wname][och])

        # -- v, produced transposed: vT[n, o] = h[:, n].T @ wvT + 1 (x) bv --
        vt = []
        for j in range(NJ):
            pv = pp.tile([P, C], F32, tag="ps")
            for cch in range(NCH):
                nc.tensor.matmul(pv, r(ht[cch][:, j * P:(j + 1) * P]), r(wt["v"][cch]),
                                 start=(cch == 0), stop=False)
            nc.tensor.matmul(pv, r(ones_row), r(bv_row), start=False, stop=True)
            v_ = pools["vtp"].tile([P, C], F32R, tag=f"vt{j}")
            nc.scalar.copy(out=v_, in_=pv)
            vt.append(v_)

        # -- attention, one i-half (512 queries) at a time --
        fin = [pools["outp"].tile([P, N], F32, name=f"fin{och}", tag=f"fin{och}") for och in range(NCH)]
        for ih in range(NIH):
            isl = slice(ih * IH, (ih + 1) * IH)
            prs = pp.tile([1, IH], F32, tag="ps")               # rowsum accum
            po = [pp.tile([P, IH], F32, name=f"po{_}", tag="ps") for _ in range(NCH)]  # outU accum
            # burst all score matmuls + exps first so the PE never stalls on
            # ACT inside the accumulation stream
            ets = []
            for j in range(NJ):
                ps = pp.tile([P, IH], F32, tag="ps")
                for cch in range(NCH):
                    nc.tensor.matmul(ps,
                                     r(kt[cch][:, j * P:(j + 1) * P]),
                                     r(qt[cch][:, isl]),
                                     start=(cch == 0), stop=(cch == NCH - 1))
                et = pools["etp"].tile([P, IH], F32R, name=f"et{j}", tag=f"et{j}")
                nc.scalar.activation(out=et, in_=ps, func=AF.Exp)
                ets.append(et)
            for j in range(NJ):
                et = ets[j]
                nc.tensor.matmul(prs, r(ones_col), r(et), start=(j == 0), stop=(j == NJ - 1))
                for och in range(NCH):
                    nc.tensor.matmul(po[och], r(vt[j][:, och * P:(och + 1) * P]), r(et),
                                     start=(j == 0), stop=(j == NJ - 1))
            rcp = sm.tile([1, IH], F32, tag="rcp")
            rscratch = sm.tile([1, IH], F32, tag="rscratch")
            nc.vector.reciprocal_approx_accurate(out=rcp, in_=prs, scratch=rscratch)
            rb = sm.tile([P, IH], F32, tag="rb")
            nc.gpsimd.partition_broadcast(rb, rcp)
            ou = []
            for cch in range(NCH):
                o_ = pools["outp"].tile([P, IH], F32R, tag=f"ou{cch}")
                nc.vector.tensor_mul(o_, po[cch], rb)           # normalize
                ou.append(o_)
            for och in range(NCH):
                pz = pp.tile([P, IH], F32, tag="ps")
                for cch in range(NCH):
                    nc.tensor.matmul(pz,
                                     r(wt["p"][cch][:, och * P:(och + 1) * P]),
                                     r(ou[cch]),
                                     start=(cch == 0), stop=(cch == NCH - 1))
                # y = (wp@ou + bp) + x   in one fused DVE pass
                nc.vector.scalar_tensor_tensor(
                    out=fin[och][:, isl], in0=pz, scalar=bt["p"][och],
                    in1=xt[och][:, isl], op0=OP.add, op1=OP.add)
        for och in range(NCH):
            nc.sync.dma_start(out=y_d[b, och * P:(och + 1) * P, :], in_=fin[och])

    for k in reversed(list(ctxpools)):
        ctxpools[k].__exit__(None, None, None)


def build_bass():
    nc = bacc.Bacc("TRN2", target_bir_lowering=False, debug=False)
    x_d = nc.dram_tensor("x", [BPC, C, N], F32, kind="ExternalInput")
    wd = {name: nc.dram_tensor(f"w{name}T", [C, C], F32R, kind="ExternalInput")
          for name in ("q", "k", "v", "p")}
    bd = {name: nc.dram_tensor(f"b{name}", [C], F32, kind="ExternalInput")
          for name in ("q", "k", "p")}
    bvr_d = nc.dram_tensor("bvr", [C], F32R, kind="ExternalInput")
    gnw_d = nc.dram_tensor("gnw", [C], F32, kind="ExternalInput")
    gnb_d = nc.dram_tensor("gnb", [C], F32, kind="ExternalInput")
    ind_d = nc.dram_tensor("ind", [C, G], F32, kind="ExternalInput")
    indT_d = nc.dram_tensor("indT", [G, C], F32, kind="ExternalInput")
    ones_d = nc.dram_tensor("ones", [P, P], F32R, kind="ExternalInput")
    y_d = nc.dram_tensor("y", [BPC, C, N], F32, kind="ExternalOutput")

    with tile.TileContext(nc) as tc:
        build_kernel_body(nc, tc, x_d, y_d, wd, bd, bvr_d, gnw_d, gnb_d, ind_d, indT_d, ones_d)
    nc.compile()
    return nc


def host_inputs(inputs):
    """Per-core replicated constants from the full input dict."""
    f = lambda a: np.ascontiguousarray(np.asarray(a), dtype=np.float32)
    scale = np.float32(C ** -0.5)
    ind = np.zeros((C, G), dtype=np.float32)
    for c in range(C):
        ind[c, c // GS] = 1.0
    consts = {
        "wqT": f(np.asarray(inputs["wq"], dtype=np.float32).T * scale),
        "bq": f(inputs["bq"]) * scale,
        "wkT": f(np.asarray(inputs["wk"], dtype=np.float32).T),
        "bk": f(inputs["bk"]),
        "wvT": f(np.asarray(inputs["wv"], dtype=np.float32).T),
        "bvr": f(inputs["bv"]),
        "wpT": f(np.asarray(inputs["wp"], dtype=np.float32).T),
        "bp": f(inputs["bp"]),
        "gnw": f(inputs["gn_w"]),
        "gnb": f(inputs["gn_b"]),
        "ind": ind,
        "indT": np.ascontiguousarray(ind.T),
        "ones": np.ones((P, P), dtype=np.float32),
    }
    return consts


_NC_CACHE = []


def _get_nc():
    if not _NC_CACHE:
        _NC_CACHE.append(build_bass())
    return _NC_CACHE[0]


def kernel(trace=False, trace_cores=None, **inputs):
    nc = _get_nc()
    consts = host_inputs(inputs)
    x = np.ascontiguousarray(np.asarray(inputs["x"], dtype=np.float32)).reshape(B, C, N)
    in_maps = []
    for core in range(NCORES):
        m = dict(consts)
        m["x"] = np.ascontiguousarray(x[core * BPC:(core + 1) * BPC])
        in_maps.append(m)
    res = run_bass_kernel_spmd(nc, in_maps, core_ids=list(range(NCORES)),
                               trace=trace, trace_cores=trace_cores)
    y = np.concatenate([r["y"] for r in res.results], axis=0)
    out = y.reshape(B, C, HH, WW).astype(np.float32)
    if trace:
        return out, res
    return out


# revision 11
# speedup vs baseline: 1.0793x; 1.0793x over previous
"""Trainium2 Bass kernel for nn_AttentionBlock (B=32, C=256, H=W=32).

Data-parallel over batch across 8 NeuronCores (4 batch elements per core);
all parameters replicated.

Algorithm per batch element (x: [C=256, N=1024]):
  h  = GroupNorm(x; 8 groups) * gn_w + gn_b
  q  = (wq/sqrt(C)) @ h + bq/sqrt(C)          [C, N]   (scale folded into wq)
  k  = wk @ h + bk                            [C, N]
  vT = hT @ wvT + 1 x bv                      [N, C]   (produced transposed!)
  ST[j,i] = sum_c k[c,j] q[c,i]               [N, N]   (scores, transposed)
  E  = exp(ST)            (scores are in [-9, 9] for this model; no max-sub)
  rowsum[i] = sum_j E[j,i]                    (ones-vector matmul, PSUM accum)
  outU[c,i] = sum_j vT[j,c] E[j,i]            (PSUM accum over j-tiles)
  y  = x + wp @ (outU * (1/rowsum)) + bp

The transposed-score formulation means no [N,N] transposes are needed:
softmax reductions over j happen on the TensorEngine partition axis via
ones/indicator matmuls, and every big matmul streams N>=256 columns in
fp32r (1 cycle/row).
"""

import numpy as np

import concourse.bacc as bacc
import concourse.bass as bass
import concourse.mybir as mybir
import concourse.tile as tile
from concourse.bass_utils import run_bass_kernel_spmd

B, C, HH, WW = 32, 256, 32, 32
N = HH * WW                 # 1024 spatial positions
NCORES = 8
BPC = B // NCORES           # batch elements per core
G = 8                       # groupnorm groups
GS = C // G                 # channels per group
P = 128                     # SBUF partitions
NCH = C // P                # channel chunks (2)
IH = 512                    # i-half width (fp32 moving-operand max)
NIH = N // IH               # 2
NJ = N // P                 # 8 j-tiles
EPS = 1e-5

F32 = mybir.dt.float32
F32R = mybir.dt.float32r
AF = mybir.ActivationFunctionType
OP = mybir.AluOpType


def r(ap):
    """fp32r APs pass straight through to the TensorEngine."""
    return ap


def build_kernel_body(nc, tc, x_d, y_d, wd, bd, bvr_d, gnw_d, gnb_d, ind_d, indT_d, ones_d):
    ctxpools = dict(
        const=tc.tile_pool(name="const", bufs=1),
        xp=tc.tile_pool(name="xp", bufs=2),
        hp=tc.tile_pool(name="hp", bufs=2),
        qk=tc.tile_pool(name="qk", bufs=2),
        vtp=tc.tile_pool(name="vtp", bufs=2),
        etp=tc.tile_pool(name="etp", bufs=2),
        sm=tc.tile_pool(name="sm", bufs=4),
        outp=tc.tile_pool(name="outp", bufs=2),
        pp=tc.tile_pool(name="pp", bufs=8, space=bass.MemorySpace.PSUM),
    )
    pools = {k: v.__enter__() for k, v in ctxpools.items()}
    const = pools["const"]
    pp = pools["pp"]
    sm = pools["sm"]

    # ---- replicated constants into SBUF ----
    wt = {}   # weights, transposed: [c_chunk][128, 256]
    bt = {}   # per-partition biases: [o_chunk][128, 1]
    for name in ("q", "k", "v", "p"):
        wt[name] = []
        for ch in range(NCH):
            w_tile = const.tile([P, C], F32R, tag=f"w{name}{ch}")
            nc.sync.dma_start(out=w_tile, in_=wd[name][ch * P:(ch + 1) * P, :])
            wt[name].append(w_tile)
    for name in ("q", "k", "p"):
        bt[name] = []
        for ch in range(NCH):
            b_tile = const.tile([P, 1], F32, tag=f"b{name}{ch}")
            nc.sync.dma_start(out=b_tile, in_=bd[name][ch * P:(ch + 1) * P][:, None])
            bt[name].append(b_tile)
    bv_row = const.tile([1, C], F32R, tag="bv_row")
    nc.sync.dma_start(out=bv_row, in_=bvr_d[None, :])

    gnw_t, gnb_t, ind_t, indT_t = [], [], [], []
    for ch in range(NCH):
        gw = const.tile([P, 1], F32, tag=f"gnw{ch}")
        nc.sync.dma_start(out=gw, in_=gnw_d[ch * P:(ch + 1) * P][:, None])
        gnw_t.append(gw)
        gb = const.tile([P, 1], F32, tag=f"gnb{ch}")
        nc.sync.dma_start(out=gb, in_=gnb_d[ch * P:(ch + 1) * P][:, None])
        gnb_t.append(gb)
        it_ = const.tile([P, G], F32, tag=f"ind{ch}")
        nc.sync.dma_start(out=it_, in_=ind_d[ch * P:(ch + 1) * P, :])
        ind_t.append(it_)
        itT = const.tile([G, P], F32, tag=f"indT{ch}")
        nc.sync.dma_start(out=itT, in_=indT_d[:, ch * P:(ch + 1) * P])
        indT_t.append(itT)

    ones_col = const.tile([P, 1], F32R, tag="ones_col")
    nc.sync.dma_start(out=ones_col, in_=ones_d[:, 0:1])
    ones_row = const.tile([1, P], F32R, tag="ones_row")
    nc.sync.dma_start(out=ones_row, in_=ones_d[0:1, :])
    eps8 = const.tile([G, 1], F32, tag="eps8")
    nc.vector.memset(eps8, EPS)

    # ---- per-batch pipeline, software-pipelined across batches ----
    st = {}   # per-batch tiles: xt, ht, qt, kt, vt, fin

    def emit_head(b):
        # load x[b] as two channel-chunk tiles [128, 1024]
        xt = []
        for ch in range(NCH):
            t = pools["xp"].tile([P, N], F32, name=f"xt{ch}", tag=f"xt{ch}")
            nc.sync.dma_start(out=t, in_=x_d[b, ch * P:(ch + 1) * P, :])
            xt.append(t)

        # -- GroupNorm statistics --
        # per-channel mean / E[x^2] over the 1024 free elements
        pcs = []
        for ch in range(NCH):
            stats = sm.tile([P, 2, 6], F32, tag="bnstats")
            for sg in range(2):
                nc.vector.bn_stats(out=stats[:, sg, :], in_=xt[ch][:, sg * 512:(sg + 1) * 512])
            mv = sm.tile([P, 2], F32, tag="mv")
            nc.vector.bn_aggr(out=mv, in_=stats)
            pc = sm.tile([P, 2], F32, tag=f"pc{ch}")
            nc.vector.tensor_copy(out=pc[:, 0:1], in_=mv[:, 0:1])
            nc.vector.scalar_tensor_tensor(out=pc[:, 1:2], in0=mv[:, 0:1],
                                           scalar=mv[:, 0:1], in1=mv[:, 1:2],
                                           op0=OP.mult, op1=OP.add)  # mean^2 + var
            pcs.append(pc)
        # group-reduce across the 32 channels of each group (partition axis)
        pg = pp.tile([G, 2], F32, tag="ps")
        for ch in range(NCH):
            nc.tensor.matmul(pg, ind_t[ch], pcs[ch], start=(ch == 0), stop=(ch == NCH - 1))
        br8 = sm.tile([G, 2], F32, tag="br8")   # [:,0]=mean_g  [:,1]=rstd_g
        nc.scalar.mul(out=br8, in_=pg, mul=1.0 / 32.0)
        m2g = sm.tile([G, 1], F32, tag="m2g")
        nc.vector.tensor_mul(m2g, br8[:, 0:1], br8[:, 0:1])
        nc.vector.tensor_sub(br8[:, 1:2], br8[:, 1:2], m2g)    # var_g
        nc.scalar.activation(out=br8[:, 1:2], in_=br8[:, 1:2], func=AF.Sqrt, bias=eps8, scale=1.0)
        nc.vector.reciprocal(out=br8[:, 1:2], in_=br8[:, 1:2])

        # broadcast group stats back to channels, fold gn affine, normalize
        ht = []
        for ch in range(NCH):
            pbc = pp.tile([P, 2], F32, tag="ps")
            nc.tensor.matmul(pbc, indT_t[ch], br8)
            s_ = sm.tile([P, 1], F32, tag=f"s{ch}")
            t_ = sm.tile([P, 1], F32, tag=f"t{ch}")
            nc.vector.tensor_mul(s_, pbc[:, 1:2], gnw_t[ch])   # s = rstd * w
            nc.vector.scalar_tensor_tensor(out=t_, in0=pbc[:, 0:1], scalar=s_,
                                           in1=gnb_t[ch], op0=OP.mult,
                                           op1=OP.subtract)    # t = mean*s - b
            h_ = pools["hp"].tile([P, N], F32R, name=f"ht{ch}", tag=f"ht{ch}")
            nc.vector.tensor_scalar(out=h_, in0=xt[ch], scalar1=s_, scalar2=t_,
                                    op0=OP.mult, op1=OP.subtract)  # x*s - t
            ht.append(h_)
        st[b] = dict(xt=xt, ht=ht)

    def emit_qkv(b):
        ht = st[b]["ht"]
        # -- q, k projections: [C, N] = W^T.T @ h (+ bias during PSUM move) --
        # i-half-major so attention on i-half 0 starts after only 4 moves
        qt = [pools["qk"].tile([P, N], F32R, name=f"qt{och}", tag=f"qt{och}")
              for och in range(NCH)]
        kt = [pools["qk"].tile([P, N], F32R, name=f"kt{och}", tag=f"kt{och}")
              for och in range(NCH)]
        for ih in range(NIH):
            for wname, dst in (("q", qt), ("k", kt)):
                for och in range(NCH):
                    pq = pp.tile([P, IH], F32, tag="ps")
                    for cch in range(NCH):
                        nc.tensor.matmul(
                            pq,
                            r(wt[wname][cch][:, och * P:(och + 1) * P]),
                            r(ht[cch][:, ih * IH:(ih + 1) * IH]),
                            start=(cch == 0), stop=(cch == NCH - 1))
                    nc.vector.tensor_scalar_add(
                        out=dst[och][:, ih * IH:(ih + 1) * IH], in0=pq,
                        scalar1=bt[wname][och])

        # -- v, produced transposed: vT[n, o] = h[:, n].T @ wvT + 1 (x) bv --
        vt = []
        for j in range(NJ):
            pv = pp.tile([P, C], F32, tag="ps")
            for cch in range(NCH):
                nc.tensor.matmul(pv, r(ht[cch][:, j * P:(j + 1) * P]), r(wt["v"][cch]),
                                 start=(cch == 0), stop=False)
            nc.tensor.matmul(pv, r(ones_row), r(bv_row), start=False, stop=True)
            v_ = pools["vtp"].tile([P, C], F32R, name=f"vt{j}", tag=f"vt{j}")
            nc.scalar.copy(out=v_, in_=pv)
            vt.append(v_)
        st[b].update(qt=qt, kt=kt, vt=vt)

    def emit_attn_ih(b, ih):
        xt, qt, kt, vt = (st[b][k] for k in ("xt", "qt", "kt", "vt"))
        if ih == 0:
            st[b]["fin"] = [pools["outp"].tile([P, N], F32, name=f"fin{och}",
                                               tag=f"fin{och}") for och in range(NCH)]
        fin = st[b]["fin"]
        isl = slice(ih * IH, (ih + 1) * IH)
        prs = pp.tile([1, IH], F32, tag="ps")               # rowsum accum
        po = [pp.tile([P, IH], F32, name=f"po{_}", tag="ps") for _ in range(NCH)]
        for j in range(NJ):
            ps = pp.tile([P, IH], F32, tag="ps")
            for cch in range(NCH):
                nc.tensor.matmul(ps,
                                 r(kt[cch][:, j * P:(j + 1) * P]),
                                 r(qt[cch][:, isl]),
                                 start=(cch == 0), stop=(cch == NCH - 1))
            et = pools["etp"].tile([P, IH], F32R, name=f"et{j}", tag=f"et{j}")
            nc.scalar.activation(out=et, in_=ps, func=AF.Exp)
            nc.tensor.matmul(prs, r(ones_col), r(et), start=(j == 0), stop=(j == NJ - 1))
            for och in range(NCH):
                nc.tensor.matmul(po[och], r(vt[j][:, och * P:(och + 1) * P]), r(et),
                                 start=(j == 0), stop=(j == NJ - 1))
        rcp = sm.tile([1, IH], F32, tag="rcp")
        rscratch = sm.tile([1, IH], F32, tag="rscratch")
        nc.vector.reciprocal_approx_accurate(out=rcp, in_=prs, scratch=rscratch)
        rb = sm.tile([P, IH], F32, tag="rb")
        nc.gpsimd.partition_broadcast(rb, rcp)
        ou = []
        for cch in range(NCH):
            o_ = pools["outp"].tile([P, IH], F32R, name=f"ou{cch}", tag=f"ou{cch}")
            nc.vector.tensor_mul(o_, po[cch], rb)           # normalize
            ou.append(o_)
        for och in range(NCH):
            pz = pp.tile([P, IH], F32, tag="ps")
            for cch in range(NCH):
                nc.tensor.matmul(pz,
                                 r(wt["p"][cch][:, och * P:(och + 1) * P]),
                                 r(ou[cch]),
                                 start=(cch == 0), stop=(cch == NCH - 1))
            # y = (wp@ou + bp) + x   in one fused DVE pass
            nc.vector.scalar_tensor_tensor(
                out=fin[och][:, isl], in0=pz, scalar=bt["p"][och],
                in1=xt[och][:, isl], op0=OP.add, op1=OP.add)

    def emit_out(b):
        for och in range(NCH):
            nc.sync.dma_start(out=y_d[b, och * P:(och + 1) * P, :],
                              in_=st[b]["fin"][och])
        del st[b]

    # interleave: next batch's head between this batch's i-halves, its qkv
    # after i-half 1 — the PE always has matmul work during DVE/ACT chains
    emit_head(0)
    emit_qkv(0)
    for b in range(BPC):
        emit_attn_ih(b, 0)
        if b + 1 < BPC:
            emit_head(b + 1)
        emit_attn_ih(b, 1)
        if b + 1 < BPC:
            emit_qkv(b + 1)
        emit_out(b)

    for k in reversed(list(ctxpools)):
        ctxpools[k].__exit__(None, None, None)


def build_bass():
    nc = bacc.Bacc("TRN2", target_bir_lowering=False, debug=False)
    x_d = nc.dram_tensor("x", [BPC, C, N], F32, kind="ExternalInput")
    wd = {name: nc.dram_tensor(f"w{name}T", [C, C], F32R, kind="ExternalInput")
          for name in ("q", "k", "v", "p")}
    bd = {name: nc.dram_tensor(f"b{name}", [C], F32, kind="ExternalInput")
          for name in ("q", "k", "p")}
    bvr_d = nc.dram_tensor("bvr", [C], F32R, kind="ExternalInput")
    gnw_d = nc.dram_tensor("gnw", [C], F32, kind="ExternalInput")
    gnb_d = nc.dram_tensor("gnb", [C], F32, kind="ExternalInput")
    ind_d = nc.dram_tensor("ind", [C, G], F32, kind="ExternalInput")
    indT_d = nc.dram_tensor("indT", [G, C], F32, kind="ExternalInput")
    ones_d = nc.dram_tensor("ones", [P, P], F32R, kind="ExternalInput")
    y_d = nc.dram_tensor("y", [BPC, C, N], F32, kind="ExternalOutput")

    with tile.TileContext(nc) as tc:
        build_kernel_body(nc, tc, x_d, y_d, wd, bd, bvr_d, gnw_d, gnb_d, ind_d, indT_d, ones_d)
    nc.compile()
    return nc


def host_inputs(inputs):
    """Per-core replicated constants from the full input dict."""
    f = lambda a: np.ascontiguousarray(np.asarray(a), dtype=np.float32)
    scale = np.float32(C ** -0.5)
    ind = np.zeros((C, G), dtype=np.float32)
    for c in range(C):
        ind[c, c // GS] = 1.0
    consts = {
        "wqT": f(np.asarray(inputs["wq"], dtype=np.float32).T * scale),
        "bq": f(inputs["bq"]) * scale,
        "wkT": f(np.asarray(inputs["wk"], dtype=np.float32).T),
        "bk": f(inputs["bk"]),
        "wvT": f(np.asarray(inputs["wv"], dtype=np.float32).T),
        "bvr": f(inputs["bv"]),
        "wpT": f(np.asarray(inputs["wp"], dtype=np.float32).T),
        "bp": f(inputs["bp"]),
        "gnw": f(inputs["gn_w"]),
        "gnb": f(inputs["gn_b"]),
        "ind": ind,
        "indT": np.ascontiguousarray(ind.T),
        "ones": np.ones((P, P), dtype=np.float32),
    }
    return consts


_NC_CACHE = []


def _get_nc():
    if not _NC_CACHE:
        _NC_CACHE.append(build_bass())
    return _NC_CACHE[0]


def kernel(trace=False, trace_cores=None, **inputs):
    nc = _get_nc()
    consts = host_inputs(inputs)
    x = np.ascontiguousarray(np.asarray(inputs["x"], dtype=np.float32)).reshape(B, C, N)
    in_maps = []
    for core in range(NCORES):
        m = dict(consts)
        m["x"] = np.ascontiguousarray(x[core * BPC:(core + 1) * BPC])
        in_maps.append(m)
    res = run_bass_kernel_spmd(nc, in_maps, core_ids=list(range(NCORES)),
                               trace=trace, trace_cores=trace_cores)
    y = np.concatenate([r["y"] for r in res.results], axis=0)
    out = y.reshape(B, C, HH, WW).astype(np.float32)
    if trace:
        return out, res
    return out


# revision 12
# speedup vs baseline: 1.1921x; 1.1046x over previous
"""Trainium2 Bass kernel for nn_AttentionBlock (B=32, C=256, H=W=32).

Data-parallel over batch across 8 NeuronCores (4 batch elements per core);
all parameters replicated.

Algorithm per batch element (x: [C=256, N=1024]):
  h  = GroupNorm(x; 8 groups) * gn_w + gn_b
  q  = (wq/sqrt(C)) @ h + bq/sqrt(C)          [C, N]   (scale folded into wq)
  k  = wk @ h + bk                            [C, N]
  vT = hT @ wvT + 1 x bv                      [N, C]   (produced transposed!)
  ST[j,i] = sum_c k[c,j] q[c,i]               [N, N]   (scores, transposed)
  E  = exp(ST)            (scores are in [-9, 9] for this model; no max-sub)
  rowsum[i] = sum_j E[j,i]                    (ones-vector matmul, PSUM accum)
  outU[c,i] = sum_j vT[j,c] E[j,i]            (PSUM accum over j-tiles)
  y  = x + wp @ (outU * (1/rowsum)) + bp

The transposed-score formulation means no [N,N] transposes are needed:
softmax reductions over j happen on the TensorEngine partition axis via
ones/indicator matmuls, and every big matmul streams N>=256 columns in
fp32r (1 cycle/row).
"""

import numpy as np

import concourse.bacc as bacc
import concourse.bass as bass
import concourse.mybir as mybir
import concourse.tile as tile
from concourse.bass_utils import run_bass_kernel_spmd

B, C, HH, WW = 32, 256, 32, 32
N = HH * WW                 # 1024 spatial positions
NCORES = 8
BPC = B // NCORES           # batch elements per core
G = 8                       # groupnorm groups
GS = C // G                 # channels per group
P = 128                     # SBUF partitions
NCH = C // P                # channel chunks (2)
IH = 512                    # i-half width (fp32 moving-operand max)
NIH = N // IH               # 2
NJ = N // P                 # 8 j-tiles
EPS = 1e-5

F32 = mybir.dt.float32
F32R = mybir.dt.float32r
BF16 = mybir.dt.bfloat16
# SIG: groupnorm output h, q/k and their weights (drives score precision)
# VAL: exp(S), vT, normalized out, wp weights (value path)
SIG_DT = BF16
VAL_DT = BF16
AF = mybir.ActivationFunctionType
OP = mybir.AluOpType


def r(ap):
    """fp32r APs pass straight through to the TensorEngine."""
    return ap


def build_kernel_body(nc, tc, x_d, y_d, wd, bd, bvr_d, gnw_d, gnb_d, ind_d, indT_d, ones_d):
    ctxpools = dict(
        const=tc.tile_pool(name="const", bufs=1),
        xp=tc.tile_pool(name="xp", bufs=2),
        hp=tc.tile_pool(name="hp", bufs=2),
        qk=tc.tile_pool(name="qk", bufs=2),
        vtp=tc.tile_pool(name="vtp", bufs=2),
        etp=tc.tile_pool(name="etp", bufs=2),
        sm=tc.tile_pool(name="sm", bufs=4),
        outp=tc.tile_pool(name="outp", bufs=2),
        pp=tc.tile_pool(name="pp", bufs=8, space=bass.MemorySpace.PSUM),
    )
    pools = {k: v.__enter__() for k, v in ctxpools.items()}
    const = pools["const"]
    pp = pools["pp"]
    sm = pools["sm"]

    # ---- replicated constants into SBUF ----
    wt = {}   # weights, transposed: [c_chunk][128, 256]
    bt = {}   # per-partition biases: [o_chunk][128, 1]
    for name in ("q", "k", "v", "p"):
        wt[name] = []
        for ch in range(NCH):
            wdt = VAL_DT if name == "p" else SIG_DT
            w_tile = const.tile([P, C], wdt, tag=f"w{name}{ch}")
            nc.sync.dma_start(out=w_tile, in_=wd[name][ch * P:(ch + 1) * P, :])
            wt[name].append(w_tile)
    for name in ("q", "k", "p"):
        bt[name] = []
        for ch in range(NCH):
            b_tile = const.tile([P, 1], F32, tag=f"b{name}{ch}")
            nc.sync.dma_start(out=b_tile, in_=bd[name][ch * P:(ch + 1) * P][:, None])
            bt[name].append(b_tile)
    bv_row = const.tile([1, C], VAL_DT, tag="bv_row")
    nc.sync.dma_start(out=bv_row, in_=bvr_d[None, :])

    gnw_t, gnb_t, ind_t, indT_t = [], [], [], []
    for ch in range(NCH):
        gw = const.tile([P, 1], F32, tag=f"gnw{ch}")
        nc.sync.dma_start(out=gw, in_=gnw_d[ch * P:(ch + 1) * P][:, None])
        gnw_t.append(gw)
        gb = const.tile([P, 1], F32, tag=f"gnb{ch}")
        nc.sync.dma_start(out=gb, in_=gnb_d[ch * P:(ch + 1) * P][:, None])
        gnb_t.append(gb)
        it_ = const.tile([P, G], F32, tag=f"ind{ch}")
        nc.sync.dma_start(out=it_, in_=ind_d[ch * P:(ch + 1) * P, :])
        ind_t.append(it_)
        itT = const.tile([G, P], F32, tag=f"indT{ch}")
        nc.sync.dma_start(out=itT, in_=indT_d[:, ch * P:(ch + 1) * P])
        indT_t.append(itT)

    ones_col = const.tile([P, 1], VAL_DT, tag="ones_col")
    nc.sync.dma_start(out=ones_col, in_=ones_d[:, 0:1])
    ones_row = const.tile([1, P], VAL_DT, tag="ones_row")
    nc.sync.dma_start(out=ones_row, in_=ones_d[0:1, :])
    eps8 = const.tile([G, 1], F32, tag="eps8")
    nc.vector.memset(eps8, EPS)

    # ---- per-batch pipeline, software-pipelined across batches ----
    st = {}   # per-batch tiles: xt, ht, qt, kt, vt, fin

    def emit_head(b):
        # load x[b] as two channel-chunk tiles [128, 1024]
        xt = []
        for ch in range(NCH):
            t = pools["xp"].tile([P, N], F32, name=f"xt{ch}", tag=f"xt{ch}")
            nc.sync.dma_start(out=t, in_=x_d[b, ch * P:(ch + 1) * P, :])
            xt.append(t)

        # -- GroupNorm statistics --
        # per-channel mean / E[x^2] over the 1024 free elements
        pcs = []
        for ch in range(NCH):
            stats = sm.tile([P, 2, 6], F32, tag="bnstats")
            for sg in range(2):
                nc.vector.bn_stats(out=stats[:, sg, :], in_=xt[ch][:, sg * 512:(sg + 1) * 512])
            mv = sm.tile([P, 2], F32, tag="mv")
            nc.vector.bn_aggr(out=mv, in_=stats)
            pc = sm.tile([P, 2], F32, tag=f"pc{ch}")
            nc.vector.tensor_copy(out=pc[:, 0:1], in_=mv[:, 0:1])
            nc.vector.scalar_tensor_tensor(out=pc[:, 1:2], in0=mv[:, 0:1],
                                           scalar=mv[:, 0:1], in1=mv[:, 1:2],
                                           op0=OP.mult, op1=OP.add)  # mean^2 + var
            pcs.append(pc)
        # group-reduce across the 32 channels of each group (partition axis)
        pg = pp.tile([G, 2], F32, tag="ps")
        for ch in range(NCH):
            nc.tensor.matmul(pg, ind_t[ch], pcs[ch], start=(ch == 0), stop=(ch == NCH - 1))
        br8 = sm.tile([G, 2], F32, tag="br8")   # [:,0]=mean_g  [:,1]=rstd_g
        nc.scalar.mul(out=br8, in_=pg, mul=1.0 / 32.0)
        m2g = sm.tile([G, 1], F32, tag="m2g")
        nc.vector.tensor_mul(m2g, br8[:, 0:1], br8[:, 0:1])
        nc.vector.tensor_sub(br8[:, 1:2], br8[:, 1:2], m2g)    # var_g
        nc.scalar.activation(out=br8[:, 1:2], in_=br8[:, 1:2], func=AF.Sqrt, bias=eps8, scale=1.0)
        nc.vector.reciprocal(out=br8[:, 1:2], in_=br8[:, 1:2])

        # broadcast group stats back to channels, fold gn affine, normalize
        ht = []
        for ch in range(NCH):
            pbc = pp.tile([P, 2], F32, tag="ps")
            nc.tensor.matmul(pbc, indT_t[ch], br8)
            s_ = sm.tile([P, 1], F32, tag=f"s{ch}")
            t_ = sm.tile([P, 1], F32, tag=f"t{ch}")
            nc.vector.tensor_mul(s_, pbc[:, 1:2], gnw_t[ch])   # s = rstd * w
            nc.vector.scalar_tensor_tensor(out=t_, in0=pbc[:, 0:1], scalar=s_,
                                           in1=gnb_t[ch], op0=OP.mult,
                                           op1=OP.subtract)    # t = mean*s - b
            h_ = pools["hp"].tile([P, N], SIG_DT, name=f"ht{ch}", tag=f"ht{ch}")
            nc.vector.tensor_scalar(out=h_, in0=xt[ch], scalar1=s_, scalar2=t_,
                                    op0=OP.mult, op1=OP.subtract)  # x*s - t
            ht.append(h_)
        st[b] = dict(xt=xt, ht=ht)

    def emit_qkv(b):
        ht = st[b]["ht"]
        # -- q, k projections: [C, N] = W^T.T @ h (+ bias during PSUM move) --
        # i-half-major so attention on i-half 0 starts after only 4 moves
        qt = [pools["qk"].tile([P, N], SIG_DT, name=f"qt{och}", tag=f"qt{och}")
              for och in range(NCH)]
        kt = [pools["qk"].tile([P, N], SIG_DT, name=f"kt{och}", tag=f"kt{och}")
              for och in range(NCH)]
        for ih in range(NIH):
            for wname, dst in (("q", qt), ("k", kt)):
                for och in range(NCH):
                    pq = pp.tile([P, IH], F32, tag="ps")
                    for cch in range(NCH):
                        nc.tensor.matmul(
                            pq,
                            r(wt[wname][cch][:, och * P:(och + 1) * P]),
                            r(ht[cch][:, ih * IH:(ih + 1) * IH]),
                            start=(cch == 0), stop=(cch == NCH - 1))
                    nc.vector.tensor_scalar_add(
                        out=dst[och][:, ih * IH:(ih + 1) * IH], in0=pq,
                        scalar1=bt[wname][och])

        # -- v, produced transposed: vT[n, o] = h[:, n].T @ wvT + 1 (x) bv --
        vt = []
        for j in range(NJ):
            pv = pp.tile([P, C], F32, tag="ps")
            for cch in range(NCH):
                nc.tensor.matmul(pv, r(ht[cch][:, j * P:(j + 1) * P]), r(wt["v"][cch]),
                                 start=(cch == 0), stop=False)
            nc.tensor.matmul(pv, r(ones_row), r(bv_row), start=False, stop=True)
            v_ = pools["vtp"].tile([P, C], VAL_DT, name=f"vt{j}", tag=f"vt{j}")
            nc.scalar.copy(out=v_, in_=pv)
            vt.append(v_)
        st[b].update(qt=qt, kt=kt, vt=vt)

    def emit_attn_ih(b, ih):
        xt, qt, kt, vt = (st[b][k] for k in ("xt", "qt", "kt", "vt"))
        if ih == 0:
            st[b]["fin"] = [pools["outp"].tile([P, N], F32, name=f"fin{och}",
                                               tag=f"fin{och}") for och in range(NCH)]
        fin = st[b]["fin"]
        isl = slice(ih * IH, (ih + 1) * IH)
        prs = pp.tile([1, IH], F32, tag="ps")               # rowsum accum
        po = [pp.tile([P, IH], F32, name=f"po{_}", tag="ps") for _ in range(NCH)]
        for j in range(NJ):
            ps = pp.tile([P, IH], F32, tag="ps")
            for cch in range(NCH):
                nc.tensor.matmul(ps,
                                 r(kt[cch][:, j * P:(j + 1) * P]),
                                 r(qt[cch][:, isl]),
                                 start=(cch == 0), stop=(cch == NCH - 1))
            et = pools["etp"].tile([P, IH], VAL_DT, name=f"et{j}", tag=f"et{j}")
            nc.scalar.activation(out=et, in_=ps, func=AF.Exp)
            nc.tensor.matmul(prs, r(ones_col), r(et), start=(j == 0), stop=(j == NJ - 1))
            for och in range(NCH):
                nc.tensor.matmul(po[och], r(vt[j][:, och * P:(och + 1) * P]), r(et),
                                 start=(j == 0), stop=(j == NJ - 1))
        rcp = sm.tile([1, IH], F32, tag="rcp")
        rscratch = sm.tile([1, IH], F32, tag="rscratch")
        nc.vector.reciprocal_approx_accurate(out=rcp, in_=prs, scratch=rscratch)
        rb = sm.tile([P, IH], F32, tag="rb")
        nc.gpsimd.partition_broadcast(rb, rcp)
        ou = []
        for cch in range(NCH):
            o_ = pools["outp"].tile([P, IH], VAL_DT, name=f"ou{cch}", tag=f"ou{cch}")
            nc.vector.tensor_mul(o_, po[cch], rb)           # normalize
            ou.append(o_)
        for och in range(NCH):
            pz = pp.tile([P, IH], F32, tag="ps")
            for cch in range(NCH):
                nc.tensor.matmul(pz,
                                 r(wt["p"][cch][:, och * P:(och + 1) * P]),
                                 r(ou[cch]),
                                 start=(cch == 0), stop=(cch == NCH - 1))
            # y = (wp@ou + bp) + x   in one fused DVE pass
            nc.vector.scalar_tensor_tensor(
                out=fin[och][:, isl], in0=pz, scalar=bt["p"][och],
                in1=xt[och][:, isl], op0=OP.add, op1=OP.add)

    def emit_out(b):
        for och in range(NCH):
            nc.sync.dma_start(out=y_d[b, och * P:(och + 1) * P, :],
                              in_=st[b]["fin"][och])
        del st[b]

    # interleave: next batch's head between this batch's i-halves, its qkv
    # after i-half 1 — the PE always has matmul work during DVE/ACT chains
    emit_head(0)
    emit_qkv(0)
    for b in range(BPC):
        emit_attn_ih(b, 0)
        if b + 1 < BPC:
            emit_head(b + 1)
        emit_attn_ih(b, 1)
        if b + 1 < BPC:
            emit_qkv(b + 1)
        emit_out(b)

    for k in reversed(list(ctxpools)):
        ctxpools[k].__exit__(None, None, None)


def build_bass():
    nc = bacc.Bacc("TRN2", target_bir_lowering=False, debug=False)
    x_d = nc.dram_tensor("x", [BPC, C, N], F32, kind="ExternalInput")
    wd = {name: nc.dram_tensor(f"w{name}T", [C, C], VAL_DT if name == "p" else SIG_DT,
                               kind="ExternalInput")
          for name in ("q", "k", "v", "p")}
    bd = {name: nc.dram_tensor(f"b{name}", [C], F32, kind="ExternalInput")
          for name in ("q", "k", "p")}
    bvr_d = nc.dram_tensor("bvr", [C], VAL_DT, kind="ExternalInput")
    gnw_d = nc.dram_tensor("gnw", [C], F32, kind="ExternalInput")
    gnb_d = nc.dram_tensor("gnb", [C], F32, kind="ExternalInput")
    ind_d = nc.dram_tensor("ind", [C, G], F32, kind="ExternalInput")
    indT_d = nc.dram_tensor("indT", [G, C], F32, kind="ExternalInput")
    ones_d = nc.dram_tensor("ones", [P, P], VAL_DT, kind="ExternalInput")
    y_d = nc.dram_tensor("y", [BPC, C, N], F32, kind="ExternalOutput")

    with tile.TileContext(nc) as tc:
        build_kernel_body(nc, tc, x_d, y_d, wd, bd, bvr_d, gnw_d, gnb_d, ind_d, indT_d, ones_d)
    nc.compile()
    return nc


def host_inputs(inputs):
    """Per-core replicated constants from the full input dict."""
    import ml_dtypes
    np_sig = np.float32 if SIG_DT != BF16 else ml_dtypes.bfloat16
    np_val = np.float32 if VAL_DT != BF16 else ml_dtypes.bfloat16
    f = lambda a: np.ascontiguousarray(np.asarray(a), dtype=np.float32)
    scale = np.float32(C ** -0.5)
    ind = np.zeros((C, G), dtype=np.float32)
    for c in range(C):
        ind[c, c // GS] = 1.0
    consts = {
        "wqT": f(np.asarray(inputs["wq"], dtype=np.float32).T * scale).astype(np_sig),
        "bq": f(inputs["bq"]) * scale,
        "wkT": f(np.asarray(inputs["wk"], dtype=np.float32).T).astype(np_sig),
        "bk": f(inputs["bk"]),
        "wvT": f(np.asarray(inputs["wv"], dtype=np.float32).T).astype(np_sig),
        "bvr": f(inputs["bv"]).astype(np_val),
        "wpT": f(np.asarray(inputs["wp"], dtype=np.float32).T).astype(np_val),
        "bp": f(inputs["bp"]),
        "gnw": f(inputs["gn_w"]),
        "gnb": f(inputs["gn_b"]),
        "ind": ind,
        "indT": np.ascontiguousarray(ind.T),
        "ones": np.ones((P, P), dtype=np_val),
    }
    return consts


_NC_CACHE = []


def _get_nc():
    if not _NC_CACHE:
        _NC_CACHE.append(build_bass())
    return _NC_CACHE[0]


def kernel(trace=False, trace_cores=None, **inputs):
    nc = _get_nc()
    consts = host_inputs(inputs)
    x = np.ascontiguousarray(np.asarray(inputs["x"], dtype=np.float32)).reshape(B, C, N)
    in_maps = []
    for core in range(NCORES):
        m = dict(consts)
        m["x"] = np.ascontiguousarray(x[core * BPC:(core + 1) * BPC])
        in_maps.append(m)
    res = run_bass_kernel_spmd(nc, in_maps, core_ids=list(range(NCORES)),
                               trace=trace, trace_cores=trace_cores)
    y = np.concatenate([r["y"] for r in res.results], axis=0)
    out = y.reshape(B, C, HH, WW).astype(np.float32)
    if trace:
        return out, res
    return out


# revision 14
# speedup vs baseline: 1.5161x; 1.2718x over previous
"""Trainium2 Bass kernel for nn_AttentionBlock (B=32, C=256, H=W=32).

Data-parallel over batch across 8 NeuronCores (4 batch elements per core);
all parameters replicated.

Algorithm per batch element (x: [C=256, N=1024]):
  h  = GroupNorm(x; 8 groups) * gn_w + gn_b
  q  = (wq/sqrt(C)) @ h + bq/sqrt(C)          [C, N]   (scale folded into wq)
  k  = wk @ h + bk                            [C, N]
  vT = hT @ wvT + 1 x bv                      [N, C]   (produced transposed!)
  ST[j,i] = sum_c k[c,j] q[c,i]               [N, N]   (scores, transposed)
  E  = exp(ST)            (scores are in [-9, 9] for this model; no max-sub)
  rowsum[i] = sum_j E[j,i]                    (ones-vector matmul, PSUM accum)
  outU[c,i] = sum_j vT[j,c] E[j,i]            (PSUM accum over j-tiles)
  y  = x + wp @ (outU * (1/rowsum)) + bp

The transposed-score formulation means no [N,N] transposes are needed:
softmax reductions over j happen on the TensorEngine partition axis via
ones/indicator matmuls, and every big matmul streams N>=256 columns in
fp32r (1 cycle/row).
"""

import numpy as np

import concourse.bacc as bacc
import concourse.bass as bass
import concourse.mybir as mybir
import concourse.tile as tile
from concourse.bass_utils import run_bass_kernel_spmd

B, C, HH, WW = 32, 256, 32, 32
N = HH * WW                 # 1024 spatial positions
NCORES = 8
BPC = B // NCORES           # batch elements per core
G = 8                       # groupnorm groups
GS = C // G                 # channels per group
P = 128                     # SBUF partitions
NCH = C // P                # channel chunks (2)
IH = 512                    # i-half width (fp32 moving-operand max)
NIH = N // IH               # 2
NJ = N // P                 # 8 j-tiles
EPS = 1e-5

F32 = mybir.dt.float32
F32R = mybir.dt.float32r
BF16 = mybir.dt.bfloat16
# SIG: groupnorm output h, q/k and their weights (drives score precision)
# VAL: exp(S), vT, normalized out, wp weights (value path)
SIG_DT = BF16
VAL_DT = BF16
AF = mybir.ActivationFunctionType
OP = mybir.AluOpType


def r(ap):
    """fp32r APs pass straight through to the TensorEngine."""
    return ap


def build_kernel_body(nc, tc, x_d, y_d, wd, bd, bvr_d, gnw_d, gnb_d, ind_d, indT_d, ones_d):
    ctxpools = dict(
        const=tc.tile_pool(name="const", bufs=1),
        xp=tc.tile_pool(name="xp", bufs=4),
        hp=tc.tile_pool(name="hp", bufs=4),
        qk=tc.tile_pool(name="qk", bufs=2),
        vtp=tc.tile_pool(name="vtp", bufs=2),
        etp=tc.tile_pool(name="etp", bufs=2),
        sm=tc.tile_pool(name="sm", bufs=4),
        outp=tc.tile_pool(name="outp", bufs=2),
        pp=tc.tile_pool(name="pp", bufs=8, space=bass.MemorySpace.PSUM),
    )
    pools = {k: v.__enter__() for k, v in ctxpools.items()}
    const = pools["const"]
    pp = pools["pp"]
    sm = pools["sm"]

    # ---- replicated constants into SBUF ----
    wt = {}   # weights, transposed: [c_chunk][128, 256]
    bt = {}   # per-partition biases: [o_chunk][128, 1]
    for name in ("q", "k", "v", "p"):
        wt[name] = []
        for ch in range(NCH):
            wdt = VAL_DT if name == "p" else SIG_DT
            w_tile = const.tile([P, C], wdt, tag=f"w{name}{ch}")
            nc.sync.dma_start(out=w_tile, in_=wd[name][ch * P:(ch + 1) * P, :])
            wt[name].append(w_tile)
    for name in ("q", "k", "p"):
        bt[name] = []
        for ch in range(NCH):
            b_tile = const.tile([P, 1], F32, tag=f"b{name}{ch}")
            nc.sync.dma_start(out=b_tile, in_=bd[name][ch * P:(ch + 1) * P][:, None])
            bt[name].append(b_tile)
    bv_row = const.tile([1, C], VAL_DT, tag="bv_row")
    nc.sync.dma_start(out=bv_row, in_=bvr_d[None, :])

    gnw_t, gnb_t, ind_t, indT_t = [], [], [], []
    for ch in range(NCH):
        gw = const.tile([P, 1], F32, tag=f"gnw{ch}")
        nc.sync.dma_start(out=gw, in_=gnw_d[ch * P:(ch + 1) * P][:, None])
        gnw_t.append(gw)
        gb = const.tile([P, 1], F32, tag=f"gnb{ch}")
        nc.sync.dma_start(out=gb, in_=gnb_d[ch * P:(ch + 1) * P][:, None])
        gnb_t.append(gb)
        it_ = const.tile([P, G], F32, tag=f"ind{ch}")
        nc.sync.dma_start(out=it_, in_=ind_d[ch * P:(ch + 1) * P, :])
        ind_t.append(it_)
        itT = const.tile([G, P], F32, tag=f"indT{ch}")
        nc.sync.dma_start(out=itT, in_=indT_d[:, ch * P:(ch + 1) * P])
        indT_t.append(itT)

    ones128 = const.tile([P, P], VAL_DT, tag="ones128")
    nc.sync.dma_start(out=ones128, in_=ones_d[:, :])
    ones_row = const.tile([1, P], VAL_DT, tag="ones_row")
    nc.sync.dma_start(out=ones_row, in_=ones_d[0:1, :])
    eps8 = const.tile([G, 1], F32, tag="eps8")
    nc.vector.memset(eps8, EPS)

    # ---- per-batch pipeline, software-pipelined across batches ----
    st = {}   # per-batch tiles: xt, ht, qt, kt, vt, fin

    def emit_head(b):
        # load x[b] as two channel-chunk tiles [128, 1024]
        xt = []
        for ch in range(NCH):
            t = pools["xp"].tile([P, N], F32, name=f"xt{ch}", tag=f"xt{ch}")
            nc.sync.dma_start(out=t, in_=x_d[b, ch * P:(ch + 1) * P, :])
            xt.append(t)

        # -- GroupNorm statistics --
        # per-channel mean / E[x^2] over the 1024 free elements
        pcs = []
        for ch in range(NCH):
            stats = sm.tile([P, 2, 6], F32, tag="bnstats")
            for sg in range(2):
                nc.vector.bn_stats(out=stats[:, sg, :], in_=xt[ch][:, sg * 512:(sg + 1) * 512])
            mv = sm.tile([P, 2], F32, tag="mv")
            nc.vector.bn_aggr(out=mv, in_=stats)
            pc = sm.tile([P, 2], F32, tag=f"pc{ch}")
            nc.vector.tensor_copy(out=pc[:, 0:1], in_=mv[:, 0:1])
            nc.vector.scalar_tensor_tensor(out=pc[:, 1:2], in0=mv[:, 0:1],
                                           scalar=mv[:, 0:1], in1=mv[:, 1:2],
                                           op0=OP.mult, op1=OP.add)  # mean^2 + var
            pcs.append(pc)
        # group-reduce across the 32 channels of each group (partition axis)
        pg = pp.tile([G, 2], F32, tag="ps")
        for ch in range(NCH):
            nc.tensor.matmul(pg, ind_t[ch], pcs[ch], start=(ch == 0), stop=(ch == NCH - 1))
        br8 = sm.tile([G, 2], F32, tag="br8")   # [:,0]=mean_g  [:,1]=rstd_g
        nc.scalar.mul(out=br8, in_=pg, mul=1.0 / 32.0)
        m2g = sm.tile([G, 1], F32, tag="m2g")
        nc.vector.tensor_mul(m2g, br8[:, 0:1], br8[:, 0:1])
        nc.vector.tensor_sub(br8[:, 1:2], br8[:, 1:2], m2g)    # var_g
        nc.scalar.activation(out=br8[:, 1:2], in_=br8[:, 1:2], func=AF.Sqrt, bias=eps8, scale=1.0)
        nc.vector.reciprocal(out=br8[:, 1:2], in_=br8[:, 1:2])

        # broadcast group stats back to channels, fold gn affine, normalize
        ht = []
        for ch in range(NCH):
            pbc = pp.tile([P, 2], F32, tag="ps")
            nc.tensor.matmul(pbc, indT_t[ch], br8)
            s_ = sm.tile([P, 1], F32, tag=f"s{ch}")
            t_ = sm.tile([P, 1], F32, tag=f"t{ch}")
            nc.vector.tensor_mul(s_, pbc[:, 1:2], gnw_t[ch])   # s = rstd * w
            nc.vector.scalar_tensor_tensor(out=t_, in0=pbc[:, 0:1], scalar=s_,
                                           in1=gnb_t[ch], op0=OP.mult,
                                           op1=OP.subtract)    # t = mean*s - b
            h_ = pools["hp"].tile([P, N], SIG_DT, name=f"ht{ch}", tag=f"ht{ch}")
            nc.vector.tensor_scalar(out=h_, in0=xt[ch], scalar1=s_, scalar2=t_,
                                    op0=OP.mult, op1=OP.subtract)  # x*s - t
            ht.append(h_)
        st[b] = dict(xt=xt, ht=ht)

    def emit_qkv(b):
        ht = st[b]["ht"]
        # -- q, k projections: [C, N] = W^T.T @ h (+ bias during PSUM move) --
        # i-half-major so attention on i-half 0 starts after only 4 moves
        qt = [pools["qk"].tile([P, N], SIG_DT, name=f"qt{och}", tag=f"qt{och}")
              for och in range(NCH)]
        kt = [pools["qk"].tile([P, N], SIG_DT, name=f"kt{och}", tag=f"kt{och}")
              for och in range(NCH)]
        for ih in range(NIH):
            for wname, dst in (("q", qt), ("k", kt)):
                for och in range(NCH):
                    pq = pp.tile([P, IH], F32, tag="ps")
                    for cch in range(NCH):
                        nc.tensor.matmul(
                            pq,
                            r(wt[wname][cch][:, och * P:(och + 1) * P]),
                            r(ht[cch][:, ih * IH:(ih + 1) * IH]),
                            start=(cch == 0), stop=(cch == NCH - 1))
                    nc.vector.tensor_scalar_add(
                        out=dst[och][:, ih * IH:(ih + 1) * IH], in0=pq,
                        scalar1=bt[wname][och])

        # -- v, produced transposed: vT[n, o] = h[:, n].T @ wvT + 1 (x) bv --
        vt = []
        for j in range(NJ):
            pv = pp.tile([P, C], F32, tag="ps")
            for cch in range(NCH):
                nc.tensor.matmul(pv, r(ht[cch][:, j * P:(j + 1) * P]), r(wt["v"][cch]),
                                 start=(cch == 0), stop=False)
            nc.tensor.matmul(pv, r(ones_row), r(bv_row), start=False, stop=True)
            v_ = pools["vtp"].tile([P, C], VAL_DT, name=f"vt{j}", tag=f"vt{j}")
            nc.scalar.copy(out=v_, in_=pv)
            vt.append(v_)
        st[b].update(qt=qt, kt=kt, vt=vt)

    def emit_attn_scores(b, ih):
        qt, kt, vt = (st[b][k] for k in ("qt", "kt", "vt"))
        if ih == 0:
            st[b]["fin"] = [pools["outp"].tile([P, N], F32, name=f"fin{och}",
                                               tag=f"fin{och}") for och in range(NCH)]
        isl = slice(ih * IH, (ih + 1) * IH)
        # rowsum replicated across all 128 partitions (all-ones stationary) so
        # the reciprocal runs wide and needs no partition broadcast
        prs = pp.tile([P, IH], F32, name="prs", tag="ps")
        po = [pp.tile([P, IH], F32, name=f"po{_}", tag="ps") for _ in range(NCH)]
        for j in range(NJ):
            ps = pp.tile([P, IH], F32, tag="ps")
            for cch in range(NCH):
                nc.tensor.matmul(ps,
                                 r(kt[cch][:, j * P:(j + 1) * P]),
                                 r(qt[cch][:, isl]),
                                 start=(cch == 0), stop=(cch == NCH - 1))
            et = pools["etp"].tile([P, IH], VAL_DT, name=f"et{j}", tag=f"et{j}")
            nc.scalar.activation(out=et, in_=ps, func=AF.Exp)
            nc.tensor.matmul(prs, r(ones128), r(et), start=(j == 0), stop=(j == NJ - 1))
            for och in range(NCH):
                nc.tensor.matmul(po[och], r(vt[j][:, och * P:(och + 1) * P]), r(et),
                                 start=(j == 0), stop=(j == NJ - 1))
        st[b][f"acc{ih}"] = (prs, po)

    def emit_attn_norm(b, ih):
        prs, po = st[b][f"acc{ih}"]
        rb = sm.tile([P, IH], F32, tag="rb")
        rscratch = sm.tile([P, IH], F32, tag="rscratch")
        nc.vector.reciprocal_approx_accurate(out=rb, in_=prs, scratch=rscratch)
        ou = []
        for cch in range(NCH):
            o_ = pools["outp"].tile([P, IH], VAL_DT, name=f"ou{cch}", tag=f"ou{cch}")
            nc.vector.tensor_mul(o_, po[cch], rb)           # normalize
            ou.append(o_)
        st[b][f"ou{ih}"] = ou

    def emit_attn_out(b, ih):
        xt, fin = st[b]["xt"], st[b]["fin"]
        ou = st[b][f"ou{ih}"]
        isl = slice(ih * IH, (ih + 1) * IH)
        for och in range(NCH):
            pz = pp.tile([P, IH], F32, tag="ps")
            for cch in range(NCH):
                nc.tensor.matmul(pz,
                                 r(wt["p"][cch][:, och * P:(och + 1) * P]),
                                 r(ou[cch]),
                                 start=(cch == 0), stop=(cch == NCH - 1))
            # y = (wp@ou + bp) + x   in one fused DVE pass
            nc.vector.scalar_tensor_tensor(
                out=fin[och][:, isl], in0=pz, scalar=bt["p"][och],
                in1=xt[och][:, isl], op0=OP.add, op1=OP.add)

    def emit_out(b):
        for och in range(NCH):
            nc.sync.dma_start(out=y_d[b, och * P:(och + 1) * P, :],
                              in_=st[b]["fin"][och])
        del st[b]

    # all groupnorm heads first (clusters ACT Sqrt table loads, frees DVE
    # early); then per batch interleave so the PE always has matmul work
    # while the DVE normalization chains run
    for b in range(BPC):
        emit_head(b)
    emit_qkv(0)
    for b in range(BPC):
        emit_attn_scores(b, 0)
        emit_attn_norm(b, 0)
        emit_attn_scores(b, 1)
        emit_attn_out(b, 0)
        emit_attn_norm(b, 1)
        if b + 1 < BPC:
            emit_qkv(b + 1)
        emit_attn_out(b, 1)
        emit_out(b)

    for k in reversed(list(ctxpools)):
        ctxpools[k].__exit__(None, None, None)


def build_bass():
    nc = bacc.Bacc("TRN2", target_bir_lowering=False, debug=False)
    x_d = nc.dram_tensor("x", [BPC, C, N], F32, kind="ExternalInput")
    wd = {name: nc.dram_tensor(f"w{name}T", [C, C], VAL_DT if name == "p" else SIG_DT,
                               kind="ExternalInput")
          for name in ("q", "k", "v", "p")}
    bd = {name: nc.dram_tensor(f"b{name}", [C], F32, kind="ExternalInput")
          for name in ("q", "k", "p")}
    bvr_d = nc.dram_tensor("bvr", [C], VAL_DT, kind="ExternalInput")
    gnw_d = nc.dram_tensor("gnw", [C], F32, kind="ExternalInput")
    gnb_d = nc.dram_tensor("gnb", [C], F32, kind="ExternalInput")
    ind_d = nc.dram_tensor("ind", [C, G], F32, kind="ExternalInput")
    indT_d = nc.dram_tensor("indT", [G, C], F32, kind="ExternalInput")
    ones_d = nc.dram_tensor("ones", [P, P], VAL_DT, kind="ExternalInput")
    y_d = nc.dram_tensor("y", [BPC, C, N], F32, kind="ExternalOutput")

    with tile.TileContext(nc) as tc:
        build_kernel_body(nc, tc, x_d, y_d, wd, bd, bvr_d, gnw_d, gnb_d, ind_d, indT_d, ones_d)
    nc.compile()
    return nc


def host_inputs(inputs):
    """Per-core replicated constants from the full input dict."""
    import ml_dtypes
    np_sig = np.float32 if SIG_DT != BF16 else ml_dtypes.bfloat16
    np_val = np.float32 if VAL_DT != BF16 else ml_dtypes.bfloat16
    f = lambda a: np.ascontiguousarray(np.asarray(a), dtype=np.float32)
    scale = np.float32(C ** -0.5)
    ind = np.zeros((C, G), dtype=np.float32)
    for c in range(C):
        ind[c, c // GS] = 1.0
    consts = {
        "wqT": f(np.asarray(inputs["wq"], dtype=np.float32).T * scale).astype(np_sig),
        "bq": f(inputs["bq"]) * scale,
        "wkT": f(np.asarray(inputs["wk"], dtype=np.float32).T).astype(np_sig),
        "bk": f(inputs["bk"]),
        "wvT": f(np.asarray(inputs["wv"], dtype=np.float32).T).astype(np_sig),
        "bvr": f(inputs["bv"]).astype(np_val),
        "wpT": f(np.asarray(inputs["wp"], dtype=np.float32).T).astype(np_val),
        "bp": f(inputs["bp"]),
        "gnw": f(inputs["gn_w"]),
        "gnb": f(inputs["gn_b"]),
        "ind": ind,
        "indT": np.ascontiguousarray(ind.T),
        "ones": np.ones((P, P), dtype=np_val),
    }
    return consts


_NC_CACHE = []


def _get_nc():
    if not _NC_CACHE:
        _NC_CACHE.append(build_bass())
    return _NC_CACHE[0]


def kernel(trace=False, trace_cores=None, **inputs):
    nc = _get_nc()
    consts = host_inputs(inputs)
    x = np.ascontiguousarray(np.asarray(inputs["x"], dtype=np.float32)).reshape(B, C, N)
    in_maps = []
    for core in range(NCORES):
        m = dict(consts)
        m["x"] = np.ascontiguousarray(x[core * BPC:(core + 1) * BPC])
        in_maps.append(m)
    res = run_bass_kernel_spmd(nc, in_maps, core_ids=list(range(NCORES)),
                               trace=trace, trace_cores=trace_cores)
    y = np.concatenate([r["y"] for r in res.results], axis=0)
    out = y.reshape(B, C, HH, WW).astype(np.float32)
    if trace:
        return out, res
    return out


# revision 15
# speedup vs baseline: 1.5312x; 1.0099x over previous
"""Trainium2 Bass kernel for nn_AttentionBlock (B=32, C=256, H=W=32).

Data-parallel over batch across 8 NeuronCores (4 batch elements per core);
all parameters replicated.

Algorithm per batch element (x: [C=256, N=1024]):
  h  = GroupNorm(x; 8 groups) * gn_w + gn_b
  q  = (wq/sqrt(C)) @ h + bq/sqrt(C)          [C, N]   (scale folded into wq)
  k  = wk @ h + bk                            [C, N]
  vT = hT @ wvT + 1 x bv                      [N, C]   (produced transposed!)
  ST[j,i] = sum_c k[c,j] q[c,i]               [N, N]   (scores, transposed)
  E  = exp(ST)            (scores are in [-9, 9] for this model; no max-sub)
  rowsum[i] = sum_j E[j,i]                    (ones-vector matmul, PSUM accum)
  outU[c,i] = sum_j vT[j,c] E[j,i]            (PSUM accum over j-tiles)
  y  = x + wp @ (outU * (1/rowsum)) + bp

The transposed-score formulation means no [N,N] transposes are needed:
softmax reductions over j happen on the TensorEngine partition axis via
ones/indicator matmuls, and every big matmul streams N>=256 columns in
fp32r (1 cycle/row).
"""

import numpy as np

import concourse.bacc as bacc
import concourse.bass as bass
import concourse.mybir as mybir
import concourse.tile as tile
from concourse.bass_utils import run_bass_kernel_spmd

B, C, HH, WW = 32, 256, 32, 32
N = HH * WW                 # 1024 spatial positions
NCORES = 8
BPC = B // NCORES           # batch elements per core
G = 8                       # groupnorm groups
GS = C // G                 # channels per group
P = 128                     # SBUF partitions
NCH = C // P                # channel chunks (2)
IH = 512                    # i-half width (fp32 moving-operand max)
NIH = N // IH               # 2
NJ = N // P                 # 8 j-tiles
EPS = 1e-5

F32 = mybir.dt.float32
F32R = mybir.dt.float32r
BF16 = mybir.dt.bfloat16
# SIG: groupnorm output h, q/k and their weights (drives score precision)
# VAL: exp(S), vT, normalized out, wp weights (value path)
SIG_DT = BF16
VAL_DT = BF16
AF = mybir.ActivationFunctionType
OP = mybir.AluOpType


def r(ap):
    """fp32r APs pass straight through to the TensorEngine."""
    return ap


def build_kernel_body(nc, tc, x_d, y_d, wd, bd, bvr_d, gnw_d, gnb_d, ind_d, indT_d, ones_d):
    ctxpools = dict(
        const=tc.tile_pool(name="const", bufs=1),
        xp=tc.tile_pool(name="xp", bufs=4),
        hp=tc.tile_pool(name="hp", bufs=4),
        qk=tc.tile_pool(name="qk", bufs=2),
        vtp=tc.tile_pool(name="vtp", bufs=2),
        etp=tc.tile_pool(name="etp", bufs=2),
        sm=tc.tile_pool(name="sm", bufs=4),
        outp=tc.tile_pool(name="outp", bufs=2),
        pp=tc.tile_pool(name="pp", bufs=8, space=bass.MemorySpace.PSUM),
    )
    pools = {k: v.__enter__() for k, v in ctxpools.items()}
    const = pools["const"]
    pp = pools["pp"]
    sm = pools["sm"]

    # ---- replicated constants into SBUF ----
    wt = {}   # weights, transposed: [c_chunk][128, 256]
    bt = {}   # per-partition biases: [o_chunk][128, 1]
    for name in ("q", "k", "v", "p"):
        wt[name] = []
        for ch in range(NCH):
            wdt = VAL_DT if name == "p" else SIG_DT
            w_tile = const.tile([P, C], wdt, tag=f"w{name}{ch}")
            nc.sync.dma_start(out=w_tile, in_=wd[name][ch * P:(ch + 1) * P, :])
            wt[name].append(w_tile)
    for name in ("q", "k", "p"):
        bt[name] = []
        for ch in range(NCH):
            b_tile = const.tile([P, 1], F32, tag=f"b{name}{ch}")
            nc.sync.dma_start(out=b_tile, in_=bd[name][ch * P:(ch + 1) * P][:, None])
            bt[name].append(b_tile)
    bv_row = const.tile([1, C], VAL_DT, tag="bv_row")
    nc.sync.dma_start(out=bv_row, in_=bvr_d[None, :])

    gnw_t, gnb_t, ind_t, indT_t = [], [], [], []
    for ch in range(NCH):
        gw = const.tile([P, 1], F32, tag=f"gnw{ch}")
        nc.sync.dma_start(out=gw, in_=gnw_d[ch * P:(ch + 1) * P][:, None])
        gnw_t.append(gw)
        gb = const.tile([P, 1], F32, tag=f"gnb{ch}")
        nc.sync.dma_start(out=gb, in_=gnb_d[ch * P:(ch + 1) * P][:, None])
        gnb_t.append(gb)
        it_ = const.tile([P, G], F32, tag=f"ind{ch}")
        nc.sync.dma_start(out=it_, in_=ind_d[ch * P:(ch + 1) * P, :])
        ind_t.append(it_)
        itT = const.tile([G, P], F32, tag=f"indT{ch}")
        nc.sync.dma_start(out=itT, in_=indT_d[:, ch * P:(ch + 1) * P])
        indT_t.append(itT)

    ones128 = const.tile([P, P], VAL_DT, tag="ones128")
    nc.sync.dma_start(out=ones128, in_=ones_d[:, :])
    ones_row = const.tile([1, P], VAL_DT, tag="ones_row")
    nc.sync.dma_start(out=ones_row, in_=ones_d[0:1, :])
    eps8 = const.tile([G, 1], F32, tag="eps8")
    nc.vector.memset(eps8, EPS)

    # ---- per-batch pipeline, software-pipelined across batches ----
    st = {}   # per-batch tiles: xt, ht, qt, kt, vt, fin

    def emit_head(b):
        # load x[b] as two channel-chunk tiles [128, 1024]
        xt = []
        for ch in range(NCH):
            t = pools["xp"].tile([P, N], F32, name=f"xt{ch}", tag=f"xt{ch}")
            nc.sync.dma_start(out=t, in_=x_d[b, ch * P:(ch + 1) * P, :])
            xt.append(t)

        # -- GroupNorm statistics --
        # per-channel mean / E[x^2] over the 1024 free elements
        pcs = []
        for ch in range(NCH):
            stats = sm.tile([P, 2, 6], F32, tag="bnstats")
            for sg in range(2):
                nc.vector.bn_stats(out=stats[:, sg, :], in_=xt[ch][:, sg * 512:(sg + 1) * 512])
            mv = sm.tile([P, 2], F32, tag="mv")
            nc.vector.bn_aggr(out=mv, in_=stats)
            pc = sm.tile([P, 2], F32, tag=f"pc{ch}")
            nc.vector.tensor_copy(out=pc[:, 0:1], in_=mv[:, 0:1])
            nc.vector.scalar_tensor_tensor(out=pc[:, 1:2], in0=mv[:, 0:1],
                                           scalar=mv[:, 0:1], in1=mv[:, 1:2],
                                           op0=OP.mult, op1=OP.add)  # mean^2 + var
            pcs.append(pc)
        # group-reduce across the 32 channels of each group (partition axis)
        pg = pp.tile([G, 2], F32, tag="ps")
        for ch in range(NCH):
            nc.tensor.matmul(pg, ind_t[ch], pcs[ch], start=(ch == 0), stop=(ch == NCH - 1))
        br8 = sm.tile([G, 2], F32, tag="br8")   # [:,0]=mean_g  [:,1]=rstd_g
        nc.scalar.mul(out=br8, in_=pg, mul=1.0 / 32.0)
        m2g = sm.tile([G, 1], F32, tag="m2g")
        nc.vector.tensor_mul(m2g, br8[:, 0:1], br8[:, 0:1])
        nc.vector.tensor_sub(br8[:, 1:2], br8[:, 1:2], m2g)    # var_g
        nc.scalar.activation(out=br8[:, 1:2], in_=br8[:, 1:2], func=AF.Sqrt, bias=eps8, scale=1.0)
        nc.vector.reciprocal(out=br8[:, 1:2], in_=br8[:, 1:2])

        # broadcast group stats back to channels, fold gn affine, normalize
        ht = []
        for ch in range(NCH):
            pbc = pp.tile([P, 2], F32, tag="ps")
            nc.tensor.matmul(pbc, indT_t[ch], br8)
            s_ = sm.tile([P, 1], F32, tag=f"s{ch}")
            t_ = sm.tile([P, 1], F32, tag=f"t{ch}")
            nc.vector.tensor_mul(s_, pbc[:, 1:2], gnw_t[ch])   # s = rstd * w
            nc.vector.scalar_tensor_tensor(out=t_, in0=pbc[:, 0:1], scalar=s_,
                                           in1=gnb_t[ch], op0=OP.mult,
                                           op1=OP.subtract)    # t = mean*s - b
            h_ = pools["hp"].tile([P, N], SIG_DT, name=f"ht{ch}", tag=f"ht{ch}")
            nc.vector.tensor_scalar(out=h_, in0=xt[ch], scalar1=s_, scalar2=t_,
                                    op0=OP.mult, op1=OP.subtract)  # x*s - t
            ht.append(h_)
        st[b] = dict(xt=xt, ht=ht)

    def emit_qkv(b):
        ht = st[b]["ht"]
        # -- q, k projections: [C, N] = W^T.T @ h (+ bias during PSUM move) --
        # i-half-major so attention on i-half 0 starts after only 4 moves
        qt = [pools["qk"].tile([P, N], SIG_DT, name=f"qt{och}", tag=f"qt{och}")
              for och in range(NCH)]
        kt = [pools["qk"].tile([P, N], SIG_DT, name=f"kt{och}", tag=f"kt{och}")
              for och in range(NCH)]
        for ih in range(NIH):
            for wname, dst in (("q", qt), ("k", kt)):
                for och in range(NCH):
                    pq = pp.tile([P, IH], F32, tag="ps")
                    for cch in range(NCH):
                        nc.tensor.matmul(
                            pq,
                            r(wt[wname][cch][:, och * P:(och + 1) * P]),
                            r(ht[cch][:, ih * IH:(ih + 1) * IH]),
                            start=(cch == 0), stop=(cch == NCH - 1))
                    if wname == "k":
                        nc.scalar.add(out=dst[och][:, ih * IH:(ih + 1) * IH],
                                      in_=pq, add=bt[wname][och])
                    else:
                        nc.vector.tensor_scalar_add(
                            out=dst[och][:, ih * IH:(ih + 1) * IH], in0=pq,
                            scalar1=bt[wname][och])

        # -- v, produced transposed: vT[n, o] = h[:, n].T @ wvT + 1 (x) bv --
        vt = []
        for j in range(NJ):
            pv = pp.tile([P, C], F32, tag="ps")
            for cch in range(NCH):
                nc.tensor.matmul(pv, r(ht[cch][:, j * P:(j + 1) * P]), r(wt["v"][cch]),
                                 start=(cch == 0), stop=False)
            nc.tensor.matmul(pv, r(ones_row), r(bv_row), start=False, stop=True)
            v_ = pools["vtp"].tile([P, C], VAL_DT, name=f"vt{j}", tag=f"vt{j}")
            nc.scalar.copy(out=v_, in_=pv)
            vt.append(v_)
        st[b].update(qt=qt, kt=kt, vt=vt)

    def emit_attn_scores(b, ih):
        qt, kt, vt = (st[b][k] for k in ("qt", "kt", "vt"))
        if ih == 0:
            st[b]["fin"] = [pools["outp"].tile([P, N], F32, name=f"fin{och}",
                                               tag=f"fin{och}") for och in range(NCH)]
        isl = slice(ih * IH, (ih + 1) * IH)
        # rowsum replicated across all 128 partitions (all-ones stationary) so
        # the reciprocal runs wide and needs no partition broadcast
        prs = pp.tile([P, IH], F32, name="prs", tag="ps")
        po = [pp.tile([P, IH], F32, name=f"po{_}", tag="ps") for _ in range(NCH)]
        for j in range(NJ):
            ps = pp.tile([P, IH], F32, tag="ps")
            for cch in range(NCH):
                nc.tensor.matmul(ps,
                                 r(kt[cch][:, j * P:(j + 1) * P]),
                                 r(qt[cch][:, isl]),
                                 start=(cch == 0), stop=(cch == NCH - 1))
            et = pools["etp"].tile([P, IH], VAL_DT, name=f"et{j}", tag=f"et{j}")
            nc.scalar.activation(out=et, in_=ps, func=AF.Exp)
            nc.tensor.matmul(prs, r(ones128), r(et), start=(j == 0), stop=(j == NJ - 1))
            for och in range(NCH):
                nc.tensor.matmul(po[och], r(vt[j][:, och * P:(och + 1) * P]), r(et),
                                 start=(j == 0), stop=(j == NJ - 1))
        st[b][f"acc{ih}"] = (prs, po)

    def emit_attn_norm(b, ih):
        prs, po = st[b][f"acc{ih}"]
        rb = sm.tile([P, IH], F32, tag="rb")
        rscratch = sm.tile([P, IH], F32, tag="rscratch")
        nc.vector.reciprocal_approx_accurate(out=rb, in_=prs, scratch=rscratch)
        ou = []
        for cch in range(NCH):
            o_ = pools["outp"].tile([P, IH], VAL_DT, name=f"ou{cch}", tag=f"ou{cch}")
            nc.vector.tensor_mul(o_, po[cch], rb)           # normalize
            ou.append(o_)
        st[b][f"ou{ih}"] = ou

    def emit_attn_out(b, ih):
        xt, fin = st[b]["xt"], st[b]["fin"]
        ou = st[b][f"ou{ih}"]
        isl = slice(ih * IH, (ih + 1) * IH)
        for och in range(NCH):
            pz = pp.tile([P, IH], F32, tag="ps")
            for cch in range(NCH):
                nc.tensor.matmul(pz,
                                 r(wt["p"][cch][:, och * P:(och + 1) * P]),
                                 r(ou[cch]),
                                 start=(cch == 0), stop=(cch == NCH - 1))
            # y = (wp@ou + bp) + x   in one fused DVE pass
            nc.vector.scalar_tensor_tensor(
                out=fin[och][:, isl], in0=pz, scalar=bt["p"][och],
                in1=xt[och][:, isl], op0=OP.add, op1=OP.add)

    def emit_out(b):
        for och in range(NCH):
            nc.sync.dma_start(out=y_d[b, och * P:(och + 1) * P, :],
                              in_=st[b]["fin"][och])
        del st[b]

    # heads hoisted early (clusters ACT Sqrt table loads, frees DVE early);
    # qkv(0) right after head(0) so the PE has work during heads 1-3
    emit_head(0)
    emit_qkv(0)
    for b in range(1, BPC):
        emit_head(b)
    for b in range(BPC):
        emit_attn_scores(b, 0)
        emit_attn_norm(b, 0)
        emit_attn_scores(b, 1)
        emit_attn_out(b, 0)
        emit_attn_norm(b, 1)
        if b + 1 < BPC:
            emit_qkv(b + 1)
        emit_attn_out(b, 1)
        emit_out(b)

    for k in reversed(list(ctxpools)):
        ctxpools[k].__exit__(None, None, None)


def build_bass():
    nc = bacc.Bacc("TRN2", target_bir_lowering=False, debug=False)
    x_d = nc.dram_tensor("x", [BPC, C, N], F32, kind="ExternalInput")
    wd = {name: nc.dram_tensor(f"w{name}T", [C, C], VAL_DT if name == "p" else SIG_DT,
                               kind="ExternalInput")
          for name in ("q", "k", "v", "p")}
    bd = {name: nc.dram_tensor(f"b{name}", [C], F32, kind="ExternalInput")
          for name in ("q", "k", "p")}
    bvr_d = nc.dram_tensor("bvr", [C], VAL_DT, kind="ExternalInput")
    gnw_d = nc.dram_tensor("gnw", [C], F32, kind="ExternalInput")
    gnb_d = nc.dram_tensor("gnb", [C], F32, kind="ExternalInput")
    ind_d = nc.dram_tensor("ind", [C, G], F32, kind="ExternalInput")
    indT_d = nc.dram_tensor("indT", [G, C], F32, kind="ExternalInput")
    ones_d = nc.dram_tensor("ones", [P, P], VAL_DT, kind="ExternalInput")
    y_d = nc.dram_tensor("y", [BPC, C, N], F32, kind="ExternalOutput")

    with tile.TileContext(nc) as tc:
        build_kernel_body(nc, tc, x_d, y_d, wd, bd, bvr_d, gnw_d, gnb_d, ind_d, indT_d, ones_d)
    nc.compile()
    return nc


def host_inputs(inputs):
    """Per-core replicated constants from the full input dict."""
    import ml_dtypes
    np_sig = np.float32 if SIG_DT != BF16 else ml_dtypes.bfloat16
    np_val = np.float32 if VAL_DT != BF16 else ml_dtypes.bfloat16
    f = lambda a: np.ascontiguousarray(np.asarray(a), dtype=np.float32)
    scale = np.float32(C ** -0.5)
    ind = np.zeros((C, G), dtype=np.float32)
    for c in range(C):
        ind[c, c // GS] = 1.0
    consts = {
        "wqT": f(np.asarray(inputs["wq"], dtype=np.float32).T * scale).astype(np_sig),
        "bq": f(inputs["bq"]) * scale,
        "wkT": f(np.asarray(inputs["wk"], dtype=np.float32).T).astype(np_sig),
        "bk": f(inputs["bk"]),
        "wvT": f(np.asarray(inputs["wv"], dtype=np.float32).T).astype(np_sig),
        "bvr": f(inputs["bv"]).astype(np_val),
        "wpT": f(np.asarray(inputs["wp"], dtype=np.float32).T).astype(np_val),
        "bp": f(inputs["bp"]),
        "gnw": f(inputs["gn_w"]),
        "gnb": f(inputs["gn_b"]),
        "ind": ind,
        "indT": np.ascontiguousarray(ind.T),
        "ones": np.ones((P, P), dtype=np_val),
    }
    return consts


_NC_CACHE = []


def _get_nc():
    if not _NC_CACHE:
        _NC_CACHE.append(build_bass())
    return _NC_CACHE[0]


def kernel(trace=False, trace_cores=None, **inputs):
    nc = _get_nc()
    consts = host_inputs(inputs)
    x = np.ascontiguousarray(np.asarray(inputs["x"], dtype=np.float32)).reshape(B, C, N)
    in_maps = []
    for core in range(NCORES):
        m = dict(consts)
        m["x"] = np.ascontiguousarray(x[core * BPC:(core + 1) * BPC])
        in_maps.append(m)
    res = run_bass_kernel_spmd(nc, in_maps, core_ids=list(range(NCORES)),
                               trace=trace, trace_cores=trace_cores)
    y = np.concatenate([r["y"] for r in res.results], axis=0)
    out = y.reshape(B, C, HH, WW).astype(np.float32)
    if trace:
        return out, res
    return out
